# revision 4
# baseline (speedup 1.0000x reference)
"""AnchorSegmentMixer Trainium2 kernel (8 NeuronCores, batch-sharded).

reference:
    energy[n] = mean(w[n]**2)                       # [B]
    ratio[n]  = clip(sqrt(energy[n]/max(energy[n+1 mod B], 1e-10)), 0.02, 50)
    mixtures  = w + ratio[:, None] * roll(w, -1, axis=0)
    returns (mixtures, targets=w)

Sharding: pure data parallel over the batch axis. Core c receives rows
[32c, 32c+32] (33 rows: 32 output rows + 1 circular halo row), computes all 33
row energies locally, and emits its 32 mixture rows. No collectives needed.

On-chip layout: each 160000-sample row is spread over the 128 SBUF partitions
as [128, 1250] (partition p holds samples [1250p, 1250(p+1))), and the whole
33-row shard stays resident in SBUF (161 KiB/partition) so HBM traffic is the
roofline minimum: read 33 rows + write 32 rows per core.
"""

import numpy as np

B = 256
S = 160000
P = 128
F = S // P            # 1250 samples per partition per row
N_CORES = 8
OUT_ROWS = B // N_CORES   # 32
ROWS = OUT_ROWS + 1       # +1 halo row
EPS = 1e-10
INV_N = 1.0 / S

_cache = {}


def _build_nc():
    from contextlib import ExitStack

    import concourse.bass as bass
    import concourse.tile as tile
    from concourse import bacc, mybir

    nc = bacc.Bacc("TRN2", target_bir_lowering=False, debug=False,
                   num_devices=N_CORES)
    f32 = mybir.dt.float32
    wv = nc.declare_dram_parameter("waveforms", [ROWS, S], f32, isOutput=False)
    out = nc.declare_dram_parameter("out", [OUT_ROWS, S], f32, isOutput=True)

    in_v = wv.ap().rearrange("r (p f) -> p r f", p=P)    # [128, 33, 1250]
    out_v = out.ap().rearrange("r (p f) -> p r f", p=P)  # [128, 32, 1250]

    with tile.TileContext(nc) as tc, ExitStack() as ctx:
        data_pool = ctx.enter_context(tc.tile_pool(name="data", bufs=1))
        scr_pool = ctx.enter_context(tc.tile_pool(name="scr", bufs=1))
        outp = ctx.enter_context(tc.tile_pool(name="outp", bufs=3))
        singles = ctx.enter_context(tc.tile_pool(name="singles", bufs=1))
        psum = ctx.enter_context(tc.tile_pool(name="psum", bufs=1, space="PSUM"))

        data = data_pool.tile([P, ROWS * F], f32)
        partials = singles.tile([P, ROWS], f32)       # per-partition sum(x^2)
        inv_n_col = singles.tile([P, 1], f32)         # 1/S for the mean matmul
        ones_row = singles.tile([1, P], f32)          # broadcast matmul lhsT
        e_sb = singles.tile([1, ROWS], f32)           # mean energies
        e_bc = singles.tile([P, ROWS], f32)           # energies on all partitions
        denom = singles.tile([P, OUT_ROWS], f32)
        ratio = singles.tile([P, OUT_ROWS], f32)      # clipped mix ratios
        sq_act = scr_pool.tile([P, F], f32, tag="sq_act")

        nc.vector.memset(inv_n_col[:], INV_N)
        nc.gpsimd.memset(ones_row[:], 1.0)

        # ---- load the shard; 3-row groups => 1.92 MB per DMA ----
        G = 3
        for g in range(0, ROWS, G):
            ge = min(g + G, ROWS)
            nc.sync.dma_start(
                out=data[:, g * F:ge * F], in_=in_v[:, g:ge, :]
            )

        # ---- per-row energy partial sums (ACT; overlaps the load phase).
        # NB: vector.tensor_tensor_reduce is NOT used — it crashes this
        # runtime (probed: NRT INTERNAL error), activation+accum_out works.
        for r in range(ROWS):
            nc.scalar.activation(
                out=sq_act[:], in_=data[:, r * F:(r + 1) * F],
                func=mybir.ActivationFunctionType.Square,
                accum_out=partials[:, r:r + 1],
            )

        # ---- mean energies: ones(1/S)^T @ partials -> [1, ROWS] ----
        e_psum = psum.tile([1, ROWS], f32, tag="e")
        nc.tensor.matmul(e_psum[:], inv_n_col[:], partials[:], start=True, stop=True)
        nc.vector.tensor_copy(e_sb[:], e_psum[:])

        # ---- broadcast energies to all partitions: ones^T @ e -> [P, ROWS] ----
        bc_psum = psum.tile([P, ROWS], f32, tag="bc")
        nc.tensor.matmul(bc_psum[:], ones_row[:], e_sb[:], start=True, stop=True)
        nc.scalar.copy(e_bc[:], bc_psum[:])

        # ---- ratio = clip(sqrt(E[r] / max(E[r+1], EPS)), 0.02, 50) ----
        nc.vector.tensor_scalar_max(denom[:], e_bc[:, 1:ROWS], EPS)
        nc.vector.reciprocal(denom[:], denom[:])
        nc.vector.tensor_mul(denom[:], e_bc[:, 0:OUT_ROWS], denom[:])
        nc.scalar.sqrt(ratio[:], denom[:])
        nc.vector.tensor_scalar(
            out=ratio[:], in0=ratio[:], scalar1=50.0, scalar2=0.02,
            op0=mybir.AluOpType.min, op1=mybir.AluOpType.max,
        )

        # ---- mix: out[r] = w[r] + ratio[r] * w[r+1], one row per tile ----
        for r in range(OUT_ROWS):
            o = outp.tile([P, F], f32, tag="o")
            nc.scalar.mul(o[:], data[:, (r + 1) * F:(r + 2) * F],
                          mul=ratio[:, r:r + 1])
            nc.vector.tensor_add(o[:], o[:], data[:, r * F:(r + 1) * F])
            nc.sync.dma_start(out=out_v[:, r, :], in_=o[:])

    nc.compile()
    return nc


def _get_nc():
    if "nc" not in _cache:
        _cache["nc"] = _build_nc()
    return _cache["nc"]


def _shard_inputs(waveforms):
    in_maps = []
    for c in range(N_CORES):
        rows = (np.arange(c * OUT_ROWS, c * OUT_ROWS + ROWS)) % B
        in_maps.append({"waveforms": np.ascontiguousarray(waveforms[rows])})
    return in_maps


def kernel(waveforms):
    from concourse.bass_utils import run_bass_kernel_spmd

    waveforms = np.asarray(waveforms, dtype=np.float32)
    nc = _get_nc()
    in_maps = _shard_inputs(waveforms)
    res = run_bass_kernel_spmd(nc, in_maps, list(range(N_CORES)))
    mixtures = np.concatenate(
        [res.results[c]["out"] for c in range(N_CORES)], axis=0
    )
    return mixtures, waveforms
